# revision 28
# baseline (speedup 1.0000x reference)
"""Decorrelated (whitening) group norm for Trainium2, 8 NeuronCores.

Problem: x (16, 64, 224, 224) f32; G=32 groups where group(channel-row r) = r % 32
(after flattening batch*channel to 1024 rows). Whitening: y = sigma^{-1/2} (x - mean)
per group, sigma the 32x32 group covariance.

v3 strategy (single NEFF, SPMD on 8 cores, data-parallel over batch):
  - Core k gets rows [128k, 128k+128) as a (128, 50176) tensor; row p is group
    p % 32. The shard is resident in SBUF as bf16 (cast once at load time on the
    ACT engine); bf16 quantization contributes ~1.1e-3 output rel-err against a
    2e-2 tolerance.
  - The PE HAM clock gate defaults to K=4/8 (1.2 GHz); sustained dense matmul
    activity raises it to 8/8 (2.4 GHz). A warmup burst bridges into pass 1 with
    no idle window, and a junk-matmul bridge covers the AllReduce gap.
  - Pass 1 (overlapped with the HBM load): chunks are processed in PAIRS: two
    128-col PE transposes plus a PE-written ones column land in one (128,257)
    PSUM tile laid out [T_even | 1 | T_odd], evicted by a single DVE copy into a
    staging tile. Each pair feeds two accumulating gram matmuls: bank A gets
    T_e^T [T_e | 1] = [gram|sums], bank B gets T_o^T [1 | T_o] = [sums|gram].
  - The warmup AllReduce is triggered as the very first work on its queues so
    the ~90us ncfw settle (CC boot + barrier + first AR) completes before the
    real (32,33) AllReduce is needed.
  - On-device 32x32 math: trace-normalize, 3 Newton-Schulz iterations give
    wm = sigma^{-1/2}; bdiag(wm) and tile(-wm@mean) are broadcast via tiny PE
    matmuls.
  - Pass 2: y = bdiag(wm) @ x_bf16 + b per 512-col chunk; evictions split
    2:1 between DVE tensor_scalar_add and ACT Identity(bias); output chunks
    fill a contiguous 12-slot f32 ring and stream to HBM as 1 MB group DMAs.
  Both passes are DMA-bound (~330 GB/s measured).
"""

import functools
import os
import sys

import numpy as np

if "/opt/trn_rl_repo" not in sys.path:
    sys.path.insert(0, "/opt/trn_rl_repo")

B, C, H, W = 16, 64, 224, 224
G = 32
EPS = 1e-5
NCORES = 8
ROWS = 128                 # per-core rows = 2 batches * 64 channels
COLS = H * W               # 50176
NS_ITERS = 2

LAST_RESULTS = None        # BassKernelResults of the most recent run (for test harness)


@functools.lru_cache(maxsize=4)
def _build(cols, ncores, warm_n=60, junk_n=150, cast_grain=2048, la_pairs=2,
           istage_bufs=4):
    import ml_dtypes

    import concourse.bass as bass  # noqa: F401
    import concourse.tile as tile
    from concourse import bacc, mybir

    f32 = mybir.dt.float32
    f16 = mybir.dt.float16
    bf16 = mybir.dt.bfloat16
    ADD = mybir.AluOpType.add
    MULT = mybir.AluOpType.mult
    AFT = mybir.ActivationFunctionType

    ntch = cols // 128                # transpose chunks (392)
    npair = ntch // 2                 # transpose pairs (196)
    nach = cols // 512                # apply chunks (98)
    ntot = 4 * cols * ncores          # elements per group, all cores
    assert ntch % 2 == 0

    nc = bacc.Bacc(
        "TRN2", target_bir_lowering=False, debug=False, num_devices=ncores
    )
    xin = nc.dram_tensor("x", [ROWS, cols], f32, kind="ExternalInput")
    # fp16 output: halves the HBM write traffic; for ~N(0,1) data the fp16
    # quantization adds ~3e-4 rel-err (gathered and upcast to f32 host-side)
    yout = nc.dram_tensor("y", [ROWS, cols], f16, kind="ExternalOutput")
    xin_ap = xin.ap()
    yout_ap = yout.ap()

    i128_d = nc.inline_tensor(np.eye(128, dtype=np.float32), name="i128c")
    i128b_d = nc.inline_tensor(
        np.eye(128).astype(ml_dtypes.bfloat16), name="i128bc"
    )
    # e4[:, 128i:128(i+1)] is the [32,128] selector that places a 32x32 block at
    # rows [32i, 32i+32) of a 128-row output.
    e4np = np.zeros((32, 512), np.float32)
    for i in range(4):
        e4np[:, 128 * i + 32 * i: 128 * i + 32 * i + 32] = np.eye(32)
    e4_d = nc.inline_tensor(e4np, name="e4c")
    p4t_d = nc.inline_tensor(
        np.tile(np.eye(32, dtype=np.float32), (1, 4)), name="p4tc"
    )
    p4_d = nc.inline_tensor(
        np.tile(np.eye(32, dtype=np.float32), (4, 1)), name="p4c"
    )
    wones_d = nc.inline_tensor(np.ones((32, 1), np.float32), name="wonesc")

    with tile.TileContext(nc) as tc:
        with (
            tc.tile_pool(name="consts", bufs=1) as consts,
            tc.tile_pool(name="xpool", bufs=1) as xpool,
            tc.tile_pool(name="smalls", bufs=1) as smalls,
            tc.tile_pool(name="warmp", bufs=1) as warmp,
            tc.tile_pool(name="psW", bufs=1, space="PSUM") as psW,
        ):
            # ---- warmup AllReduce on an inline DRAM constant: no input deps,
            # so the trigger fires as soon as the CC path boots and the ~90us
            # ncfw settle (boot + barrier + first AR) hides under pass 1.
            with tc.tile_pool(name="dramw", bufs=1, space="DRAM") as dramw:
                wout = dramw.tile([32, 1], f32, name="cc_wout")
                nc.gpsimd.collective_compute(
                    "AllReduce",
                    ADD,
                    replica_groups=[list(range(ncores))],
                    ins=[wones_d.ap()],
                    outs=[wout.opt()],
                )
            ones32f = consts.tile([32, 1], f32, name="ones32f")
            nc.vector.memset(ones32f[:], 1.0)

            # ---- constants (small DMAs, before the big loads) ----
            i128 = consts.tile([128, 128], f32, name="i128")
            nc.sync.dma_start(i128[:], i128_d.ap())
            i128b = consts.tile([128, 128], bf16, name="i128b")
            nc.sync.dma_start(i128b[:], i128b_d.ap())
            e4 = consts.tile([32, 512], f32, name="e4")
            nc.sync.dma_start(e4[:], e4_d.ap())
            p4t = consts.tile([32, 128], f32, name="p4t")
            nc.sync.dma_start(p4t[:], p4t_d.ap())
            p4 = consts.tile([128, 32], f32, name="p4")
            nc.sync.dma_start(p4[:], p4_d.ap())

            # ---- HAM warmup: dense same-weight matmul burst from t~0 ----
            wsrc = warmp.tile([128, 128], bf16, name="wsrc")
            nc.vector.memset(wsrc[:], 0.0)
            wps = psW.tile([128, 128], f32, name="wps")
            for i in range(warm_n):
                nc.tensor.matmul(
                    wps[:], wsrc[:], wsrc[:],
                    start=(i == 0), stop=(i == warm_n - 1),
                )

            # resident bf16 shard
            xres = xpool.tile([128, cols], bf16, name="xres")

            o32 = consts.tile([32, 32], f32, name="o32")
            nc.vector.memset(o32[:], 1.0 / 32.0)
            c15I = consts.tile([32, 32], f32, name="c15I")
            nc.vector.tensor_scalar_mul(c15I[:], i128[0:32, 0:32], 1.5)
            epsI = consts.tile([32, 32], f32, name="epsI")
            nc.vector.tensor_scalar_mul(epsI[:], i128[0:32, 0:32], EPS)

            # preload both ACT tables off the critical path (Sqrt is used in
            # the stats math, Identity in pass-2 evictions)
            tdum = consts.tile([1, 1], f32, name="tdum")
            nc.scalar.activation(tdum[:], ones32f[0:1, :], AFT.Sqrt)
            nc.scalar.activation(
                tdum[:], ones32f[0:1, :], AFT.Identity, bias=tdum[:], scale=1.0
            )

            # ---- pass 1: load f32 -> cast bf16 resident -> gram; row sums
            # come for free from the casts' accum_out.
            # small head so compute starts early; tapered tail so the last
            # casts/transposes trail the final DMA bytes by ~1us, not ~10us
            load_sizes = ([256, 256, 512, 1024] + [4096] * 11
                          + [1024, 1024, 512, 256, 256])
            assert sum(load_sizes) == cols
            ncasts = sum(-(-sz // cast_grain) for sz in load_sizes)
            sums_t = smalls.tile([128, ncasts], f32, name="sums_t")
            with (
                tc.tile_pool(name="istage", bufs=istage_bufs) as istage,
                tc.tile_pool(name="psA", bufs=4, space="PSUM") as psA,
                tc.tile_pool(name="psGA", bufs=1, space="PSUM") as psGA,
                tc.tile_pool(name="psGB", bufs=1, space="PSUM") as psGB,
                tc.tile_pool(name="tstage", bufs=4) as tstage,
            ):
                gramA = psGA.tile([128, 128], f32, name="gramA")
                gramB = psGB.tile([128, 128], f32, name="gramB")

                pos = 0
                ci = 0
                for sz in load_sizes:
                    st = istage.tile([128, 4096], f32, name="ist")
                    nc.sync.dma_start(st[:, 0:sz], xin_ap[:, pos:pos + sz])
                    off = 0
                    while off < sz:
                        g = min(cast_grain, sz - off)
                        nc.scalar.activation(
                            xres[:, pos + off:pos + off + g],
                            st[:, off:off + g],
                            AFT.Copy,
                            accum_out=sums_t[:, ci:ci + 1],
                        )
                        ci += 1
                        off += g
                    pos += sz
                assert ci == ncasts

                tb2s = [None] * npair

                def emit_t(p):
                    # [T_even | T_odd] in one PSUM tile, one eviction
                    pt = psA.tile([128, 256], bf16, name="pt")
                    c0 = p * 256
                    nc.tensor.transpose(
                        pt[:, 0:128], xres[:, c0:c0 + 128], i128b[:]
                    )
                    nc.tensor.transpose(
                        pt[:, 128:256], xres[:, c0 + 128:c0 + 256], i128b[:]
                    )
                    tb2 = tstage.tile([128, 256], bf16, name="tb2")
                    nc.vector.tensor_copy(tb2[:], pt[:])
                    tb2s[p] = tb2

                def emit_g(p):
                    tb2 = tb2s[p]
                    nc.tensor.matmul(
                        gramA[:], tb2[:, 0:128], tb2[:, 0:128],
                        start=(p == 0), stop=(p == npair - 1),
                    )
                    nc.tensor.matmul(
                        gramB[:], tb2[:, 128:256], tb2[:, 128:256],
                        start=(p == 0), stop=(p == npair - 1),
                    )
                    tb2s[p] = None

                la = min(la_pairs, npair)
                for p in range(npair):
                    emit_t(p)
                    if p >= la:
                        emit_g(p - la)
                for p in range(npair - la, npair):
                    emit_g(p)

                gram_sbA = smalls.tile([128, 128], f32, name="gram_sbA")
                nc.vector.tensor_copy(gram_sbA[:], gramA[:])
                gram_sbB = smalls.tile([128, 128], f32, name="gram_sbB")
                nc.scalar.copy(gram_sbB[:], gramB[:])
                rsum = smalls.tile([128, 1], f32, name="rsum")
                nc.vector.reduce_sum(
                    rsum[:], sums_t[:], axis=mybir.AxisListType.X
                )

            # ---- block reduce to (32,33) + allreduce + 32x32 math ----
            with (
                tc.tile_pool(name="psS", bufs=3, space="PSUM") as psS,
                tc.tile_pool(name="mids", bufs=1) as mids,
            ):
                # sigma partial (4 diagonal 32-blocks from both banks) and the
                # group sums share one PSUM tile -> single eviction
                partQ = psS.tile([32, 33], f32, name="psml")
                for gsb_i, gsb in enumerate((gram_sbA, gram_sbB)):
                    for i in range(4):
                        nc.tensor.matmul(
                            partQ[:, 0:32],
                            i128[:, 32 * i:32 * (i + 1)],
                            gsb[:, 32 * i:32 * (i + 1)],
                            start=(gsb_i == 0 and i == 0),
                            stop=(gsb_i == 1 and i == 3),
                        )
                nc.tensor.matmul(
                    partQ[:, 32:33], p4[:], rsum[:], start=True, stop=True
                )

                part = mids.tile([32, 33], f32, name="part")
                nc.vector.tensor_copy(part[:], partQ[:])

                with tc.tile_pool(name="dram", bufs=1, space="DRAM") as dram:
                    cin = dram.tile([32, 33], f32, name="cc_in")
                    cout = dram.tile([32, 33], f32, name="cc_out")
                    nc.sync.dma_start(cin[:], part[:])
                    nc.gpsimd.collective_compute(
                        "AllReduce",
                        ADD,
                        replica_groups=[list(range(ncores))],
                        ins=[cin.opt()],
                        outs=[cout.opt()],
                    )
                    ar = mids.tile([32, 33], f32, name="ar")
                    nc.sync.dma_start(ar[:], cout[:])

                # junk bridge: keeps the PE HAM-warm across the AllReduce gap
                for i in range(junk_n):
                    nc.tensor.matmul(
                        wps[:], wsrc[:], wsrc[:],
                        start=(i == 0), stop=(i == junk_n - 1),
                    )

                # ---- tiny stats math ----
                inv_n = 1.0 / float(ntot)
                mean = mids.tile([32, 1], f32, name="mean")
                nc.vector.tensor_scalar_mul(mean[:], ar[:, 32:33], inv_n)
                sig0 = mids.tile([32, 32], f32, name="sig0")
                nc.vector.tensor_scalar_mul(sig0[:], ar[:, 0:32], inv_n)

                mrowP = psS.tile([1, 32], f32, name="psml")
                nc.tensor.transpose(mrowP[:], mean[:], i128[0:32, 0:32])
                mrow = mids.tile([1, 32], f32, name="mrow")
                nc.vector.tensor_copy(mrow[:], mrowP[:])
                outerP = psS.tile([32, 32], f32, name="psml")
                nc.tensor.matmul(outerP[:], mrow[:], mrow[:], start=True, stop=True)

                sigma = mids.tile([32, 32], f32, name="sigma")
                nc.vector.scalar_tensor_tensor(
                    out=sigma[:], in0=outerP[:], scalar=-1.0, in1=sig0[:],
                    op0=MULT, op1=ADD,
                )
                nc.vector.tensor_add(sigma[:], sigma[:], epsI[:])

                # t = trace(sigma)/32: diag-extract with fused row-reduce, then
                # one matmul against ones/32 broadcasts t to all 32 partitions
                diag = mids.tile([32, 32], f32, name="diag")
                dvec = mids.tile([32, 1], f32, name="dvec")
                nc.vector.scalar_tensor_tensor(
                    out=diag[:], in0=sigma[:], scalar=1.0,
                    in1=i128[0:32, 0:32],
                    op0=MULT, op1=MULT, accum_out=dvec[:],
                )
                tAP = psS.tile([32, 1], f32, name="psml")
                nc.tensor.matmul(tAP[:], o32[:], dvec[:], start=True, stop=True)
                rt = mids.tile([32, 1], f32, name="rt")
                nc.vector.reciprocal(rt[:], tAP[:])
                rs = mids.tile([32, 1], f32, name="rs")
                nc.scalar.activation(rs[:], rt[:], AFT.Sqrt)

                A = mids.tile([32, 32], f32, name="A")
                nc.vector.tensor_scalar_mul(A[:], sigma[:], rt[:])

                # ---- Newton-Schulz (ping-pong, no per-iter copies) ----
                Ys = [mids.tile([32, 32], f32, name=f"Y{i}") for i in range(2)]
                Zs = [mids.tile([32, 32], f32, name=f"Z{i}") for i in range(2)]
                nc.vector.tensor_copy(Ys[0][:], A[:])
                nc.vector.tensor_copy(Zs[0][:], i128[0:32, 0:32])
                with tc.tile_pool(name="nsbuf", bufs=2) as nsbuf:
                    for it in range(NS_ITERS):
                        a, b2 = it % 2, (it + 1) % 2
                        zyP = psS.tile([32, 32], f32, name="psml")
                        nc.tensor.matmul(
                            zyP[:], Zs[a][:], Ys[a][:], start=True, stop=True
                        )
                        Wt = nsbuf.tile([32, 32], f32, name="Wt")
                        nc.vector.scalar_tensor_tensor(
                            out=Wt[:], in0=zyP[:], scalar=-0.5, in1=c15I[:],
                            op0=MULT, op1=ADD,
                        )
                        ypP = psS.tile([32, 32], f32, name="psml")
                        nc.tensor.matmul(ypP[:], Ys[a][:], Wt[:], start=True, stop=True)
                        zpP = psS.tile([32, 32], f32, name="psml")
                        nc.tensor.matmul(zpP[:], Wt[:], Zs[a][:], start=True, stop=True)
                        nc.vector.tensor_copy(Ys[b2][:], ypP[:])
                        nc.scalar.copy(Zs[b2][:], zpP[:])
                Zf = Zs[NS_ITERS % 2]

                # wm = Z / sqrt(t); b = -wm @ mean
                wm = mids.tile([32, 32], f32, name="wm")
                nc.vector.tensor_scalar_mul(wm[:], Zf[:], rs[:])
                bP = psS.tile([32, 1], f32, name="psml")
                nc.tensor.matmul(bP[:], wm[:], mean[:], start=True, stop=True)
                negb = mids.tile([32, 1], f32, name="negb")
                nc.vector.tensor_scalar_mul(negb[:], bP[:], -1.0)

                # bdiag(wm) via 4 selector matmuls; tile(negb) via 1 matmul
                wm4P = psS.tile([128, 128], f32, name="psml")
                for i in range(4):
                    nc.tensor.matmul(
                        wm4P[:, 32 * i:32 * (i + 1)],
                        e4[:, 128 * i:128 * (i + 1)],
                        wm[:],
                        start=True, stop=True,
                    )
                wm4b = smalls.tile([128, 128], bf16, name="wm4b")
                nc.vector.tensor_copy(wm4b[:], wm4P[:])
                b4P = psS.tile([128, 1], f32, name="psml")
                nc.tensor.matmul(b4P[:], p4t[:], negb[:], start=True, stop=True)
                b4 = smalls.tile([128, 1], f32, name="b4")
                nc.vector.tensor_copy(b4[:], b4P[:])

            # ---- pass 2: y = bdiag(wm) @ x_bf16 + b, 1 MB group DMAs ----
            with (
                tc.tile_pool(name="psY", bufs=4, space="PSUM") as psY,
                tc.tile_pool(name="oring", bufs=1) as orp,
            ):
                og = orp.tile([128, 6144], f16, name="og")  # 12 slots, 3 groups
                for c in range(nach):
                    c0 = c * 512
                    s0 = (c % 12) * 512
                    yP = psY.tile([128, 512], f32, name="yP")
                    nc.tensor.matmul(
                        yP[:], wm4b[:], xres[:, c0:c0 + 512],
                        start=True, stop=True,
                    )
                    if c % 7 in (1, 3, 5):
                        nc.scalar.activation(
                            og[:, s0:s0 + 512], yP[:], AFT.Identity,
                            bias=b4[:], scale=1.0,
                        )
                    else:
                        nc.vector.tensor_scalar_add(
                            og[:, s0:s0 + 512], yP[:], b4[:]
                        )
                    if c < 8:
                        # per-chunk DMAs while the pipeline ramps: bytes start
                        # flowing before the first full group is evicted
                        nc.sync.dma_start(
                            yout_ap[:, c0:c0 + 512], og[:, s0:s0 + 512]
                        )
                    elif (c - 8) % 4 == 3:
                        g0 = (c - 3) * 512
                        gs = ((c - 3) % 12) * 512
                        nc.sync.dma_start(
                            yout_ap[:, g0:g0 + 2048], og[:, gs:gs + 2048]
                        )
                rem = (nach - 8) % 4
                if rem:
                    g0 = (nach - rem) * 512
                    gs = ((nach - rem) % 12) * 512
                    nc.sync.dma_start(
                        yout_ap[:, g0:g0 + rem * 512],
                        og[:, gs:gs + rem * 512],
                    )

    nc.compile()
    return nc


def _ensure_ntff_hook():
    """Register the axon NTFF profiling hook if the image's antenv lacks it."""
    try:
        import antenv.axon_hooks  # noqa: F401
        return
    except ImportError:
        pass
    try:
        import types

        import antenv
        from trn_agent_boot.trn_boot import _ntff_profile_via_ctypes

        hook = _ntff_profile_via_ctypes("/opt/axon/libaxon_pjrt.so")
        mod = types.ModuleType("antenv.axon_hooks")
        mod.get_axon_ntff_profile_hook = lambda: hook
        mod.set_axon_ntff_profile_hook = lambda h: None
        sys.modules["antenv.axon_hooks"] = mod
        antenv.axon_hooks = mod
    except Exception as e:  # profiling is best-effort
        print(f"ntff hook setup failed: {e}", file=sys.stderr)


def _run(x_flat, cols, ncores, trace=False, **build_kw):
    from concourse.bass_utils import run_bass_kernel_spmd

    if trace:
        _ensure_ntff_hook()

    nc = _build(cols, ncores, **build_kw)
    in_maps = [
        {"x": np.ascontiguousarray(x_flat[ROWS * k:ROWS * (k + 1)])}
        for k in range(ncores)
    ]
    res = run_bass_kernel_spmd(
        nc, in_maps, core_ids=list(range(ncores)), trace=trace
    )
    global LAST_RESULTS
    LAST_RESULTS = res
    return np.concatenate(
        [np.asarray(r["y"]) for r in res.results], axis=0
    ).astype(np.float32)


def kernel(x: np.ndarray) -> np.ndarray:
    x = np.asarray(x)
    assert x.shape == (B, C, H, W) and x.dtype == np.float32
    xf = x.reshape(B * C, COLS)
    trace = bool(os.environ.get("DBN_TRACE"))
    yf = _run(xf, COLS, NCORES, trace=trace)
    return yf.reshape(B, C, H, W)


if __name__ == "__main__":
    xs = np.load("/tmp/ref_in.npy")
    ys = kernel(xs)
    expected = np.load("/tmp/ref_out.npy")
    rel = np.linalg.norm(ys - expected) / np.linalg.norm(expected)
    print("fro_rel:", rel)
    if LAST_RESULTS is not None:
        print("exec_time_ns:", LAST_RESULTS.exec_time_ns)
